# revision 10
# baseline (speedup 1.0000x reference)
"""Trainium2 Bass kernel for a 2-layer ALIF spiking RNN (DoubleALIFRNN).

Strategy: data-parallel over batch (8 cores x 16 samples). Per core:
  Phase A: precompute P1 = x @ (W1[:I] * (1-alpha1)) as an fp32 GEMM
           (exact), stored feature-major in DRAM.
  Phase B: serial scan over T steps, everything SBUF-resident.
    - Feature-major layout: [features -> partitions, batch -> free].
    - Recurrent matmuls use an exact fp16 hi/lo weight split: spikes are
      binary so z @ W_hi and (z * 2^-12) @ W_lo_scaled accumulate the
      full-precision product into one fp32 PSUM group (error ~2^-24).
    - ALIF updates on DVE/ScalarE; decay constants folded into weights;
      off-critical terms precomputed so the post-matmul dependency chain
      is short.
"""
import sys

sys.path.insert(0, "/opt/trn_rl_repo")

import numpy as np
import concourse.bass as bass
import concourse.bacc as bacc
import concourse.mybir as mybir
import concourse.tile as tile
from concourse import masks
from concourse.bass_utils import run_bass_kernel_spmd

F32 = mybir.dt.float32
F16 = mybir.dt.float16

T_FULL, B_FULL, I, H1, H2, O = 2048, 128, 256, 512, 512, 128
NCORES = 8
BC = B_FULL // NCORES  # batch per core = 16
B0 = 0.01
BETA = 1.8
LO = 2.0 ** -12
C1, C2 = H1 // 128, H2 // 128   # 4, 4
KC2 = (H1 + H2) // 128          # 8
AluOp = mybir.AluOpType
ACT_COPY = mybir.ActivationFunctionType.Copy


def split_fp16(w):
    """w ~= hi + 2^-12 * lo with hi, lo fp16, exact to ~2^-24 relative.

    fp16-subnormal hi values are forced to zero so HW subnormal flushing
    cannot change the result; the lo part then carries the value exactly."""
    w = np.asarray(w, np.float32)
    hi = w.astype(np.float16)
    hi[np.abs(w) < 1.5e-4] = np.float16(0.0)
    lo = ((w - hi.astype(np.float32)) * np.float32(1.0 / LO)).astype(np.float16)
    return hi, lo


def build_program(T, BLK=8, RT_UN=8, a_unroll=None, repsA=1, repsB=1):
    """Returns (nc, meta). T = timesteps. BLK = steps per half-block.
    repsA/repsB repeat phase A / phase B for differential timing only."""
    nc = bacc.Bacc("TRN2", target_bir_lowering=False, debug=False,
                   num_devices=NCORES)
    tc_rows = T * BC
    RT = tc_rows // 128            # phase-A row tiles (8 steps each)
    NBLK = T // BLK
    RT_UN = min(RT_UN, RT)
    assert RT % RT_UN == 0 and NBLK % 2 == 0 and (BLK * BC) % 128 == 0
    OTR = BLK * BC // 128
    PW = BLK * BC

    def dram(name, shape, dtype, kind):
        return nc.dram_tensor(name, shape, dtype, kind=kind).ap()

    x_rows = dram("x_rows", [tc_rows, I], F32, "ExternalInput")
    w1x = dram("w1x", [I, H1], F32, "ExternalInput")
    w1h_hi = dram("w1h_hi", [H1, H1], F16, "ExternalInput")
    w1h_lo = dram("w1h_lo", [H1, H1], F16, "ExternalInput")
    w2_hi = dram("w2_hi", [H1 + H2, H2], F16, "ExternalInput")
    w2_lo = dram("w2_lo", [H1 + H2, H2], F16, "ExternalInput")
    wo_hi = dram("wo_hi", [H2, O], F16, "ExternalInput")
    wo_lo = dram("wo_lo", [H2, O], F16, "ExternalInput")
    al1_bc = dram("al1_bc", [128, C1 * BC], F32, "ExternalInput")
    rh1_bc = dram("rh1_bc", [128, C1 * BC], F32, "ExternalInput")
    rm1_bc = dram("rm1_bc", [128, C1 * BC], F32, "ExternalInput")
    al2_bc = dram("al2_bc", [128, C2 * BC], F32, "ExternalInput")
    rh2_bc = dram("rh2_bc", [128, C2 * BC], F32, "ExternalInput")
    rm2_bc = dram("rm2_bc", [128, C2 * BC], F32, "ExternalInput")
    ao_pp = dram("ao_pp", [128, 1], F32, "ExternalInput")

    p1d = dram("p1d", [C1, 128, tc_rows + PW], F32, "Internal")
    out_rows = dram("out_rows", [tc_rows, O], F32, "ExternalOutput")
    fin = {name: dram(name, [BC, n], F32, "ExternalOutput")
           for name, n in [("z1f", H1), ("u1f", H1), ("a1f", H1),
                           ("z2f", H2), ("u2f", H2), ("a2f", H2), ("uof", O)]}

    TT = nc.vector.tensor_tensor
    STT = nc.vector.scalar_tensor_tensor

    with tile.TileContext(nc) as tc:
        with tc.tile_pool(name="persist", bufs=1) as pp:
            ident = pp.tile([128, 128], F32)
            masks.make_identity(nc, ident[:])

            w1x_t = pp.tile([128, 2 * H1], F32)
            w1h_hi_t = pp.tile([128, 4 * H1], F16)
            w1h_lo_t = pp.tile([128, 4 * H1], F16)
            w2_hi_t = pp.tile([128, 8 * H2], F16)
            w2_lo_t = pp.tile([128, 8 * H2], F16)
            wo_hi_t = pp.tile([128, 4 * O], F16)
            wo_lo_t = pp.tile([128, 4 * O], F16)
            for k in range(2):
                nc.sync.dma_start(w1x_t[:, k * H1:(k + 1) * H1],
                                  w1x[k * 128:(k + 1) * 128, :])
            for k in range(4):
                nc.sync.dma_start(w1h_hi_t[:, k * H1:(k + 1) * H1],
                                  w1h_hi[k * 128:(k + 1) * 128, :])
                nc.sync.dma_start(w1h_lo_t[:, k * H1:(k + 1) * H1],
                                  w1h_lo[k * 128:(k + 1) * 128, :])
                nc.sync.dma_start(wo_hi_t[:, k * O:(k + 1) * O],
                                  wo_hi[k * 128:(k + 1) * 128, :])
                nc.sync.dma_start(wo_lo_t[:, k * O:(k + 1) * O],
                                  wo_lo[k * 128:(k + 1) * 128, :])
            for k in range(8):
                nc.sync.dma_start(w2_hi_t[:, k * H2:(k + 1) * H2],
                                  w2_hi[k * 128:(k + 1) * 128, :])
                nc.sync.dma_start(w2_lo_t[:, k * H2:(k + 1) * H2],
                                  w2_lo[k * 128:(k + 1) * 128, :])

            al1 = pp.tile([128, C1 * BC], F32)
            rh1 = pp.tile([128, C1 * BC], F32)
            rm1 = pp.tile([128, C1 * BC], F32)
            al2 = pp.tile([128, C2 * BC], F32)
            rh2 = pp.tile([128, C2 * BC], F32)
            rm2 = pp.tile([128, C2 * BC], F32)
            aop = pp.tile([128, 1], F32)
            for t_, d_ in [(al1, al1_bc), (rh1, rh1_bc), (rm1, rm1_bc),
                           (al2, al2_bc), (rh2, rh2_bc), (rm2, rm2_bc),
                           (aop, ao_pp)]:
                nc.sync.dma_start(t_[:], d_[:, :])

            z1 = pp.tile([128, C1 * BC], F32)
            z1h = pp.tile([128, C1 * BC], F16)
            z1s = pp.tile([128, C1 * BC], F16)
            u1 = pp.tile([128, C1 * BC], F32)
            a1 = pp.tile([128, C1 * BC], F32)
            th1 = pp.tile([128, C1 * BC], F32)
            tu1 = pp.tile([128, C1 * BC], F32)
            tm1 = pp.tile([128, C1 * BC], F32)
            z2 = pp.tile([128, C2 * BC], F32)
            z2h = pp.tile([128, C2 * BC], F16)
            z2s = pp.tile([128, C2 * BC], F16)
            u2 = pp.tile([128, C2 * BC], F32)
            a2 = pp.tile([128, C2 * BC], F32)
            th2 = pp.tile([128, C2 * BC], F32)
            tu2 = pp.tile([128, C2 * BC], F32)
            tm2 = pp.tile([128, C2 * BC], F32)
            uo = pp.tile([128, BC], F32)
            for t_ in (z1, u1, a1, z2, u2, a2, uo, z1h, z1s, z2h, z2s):
                nc.vector.memset(t_[:], 0.0)

            # zero-fill p1d's slack block so the final (wasted) prefetch
            # reads initialized memory
            zslack = pp.tile([128, PW], F32)
            nc.vector.memset(zslack[:], 0.0)
            for c in range(C1):
                nc.sync.dma_start(p1d[c, :, bass.ds(tc_rows, PW)], zslack[:])

            # ---------------- phase A ----------------
            with tc.tile_pool(name="pha", bufs=2) as pa, \
                 tc.tile_pool(name="phaps", bufs=4, space="PSUM") as paps, \
                 tc.For_i(0, repsA, 1, name="preArep") as _ra, \
                 tc.For_i(0, RT // RT_UN, 1, name="preA") as r0:
                if True:
                    for j in range(RT_UN):
                        xrow = pa.tile([128, I], F32, tag="xrow")
                        nc.sync.dma_start(
                            xrow[:], x_rows[bass.ds(r0 * (RT_UN * 128) + j * 128, 128), :])
                        xT = pa.tile([128, I], F32, tag="xT")
                        for k in range(2):
                            pst = paps.tile([128, 128], F32, tag="pst")
                            nc.tensor.transpose(pst[:], xrow[:, k * 128:(k + 1) * 128],
                                                ident[:])
                            nc.vector.tensor_copy(xT[:, k * 128:(k + 1) * 128], pst[:])
                        for m in range(C1):
                            psm = paps.tile([128, 128], F32, tag="psm")
                            for k in range(2):
                                nc.tensor.matmul(
                                    psm[:],
                                    w1x_t[:, k * H1 + m * 128:k * H1 + (m + 1) * 128],
                                    xT[:, k * 128:(k + 1) * 128],
                                    start=(k == 0), stop=(k == 1))
                            p1sb = pa.tile([128, 128], F32, tag="p1sb")
                            nc.vector.tensor_copy(p1sb[:], psm[:])
                            nc.sync.dma_start(
                                p1d[m, :, bass.ds(r0 * (RT_UN * 128) + j * 128, 128)],
                                p1sb[:])

            # ---------------- phase B ----------------
            with tc.tile_pool(name="phb", bufs=1) as pb, \
                 tc.tile_pool(name="phbps", bufs=2, space="PSUM") as bps:
                pbuf = [pb.tile([128, C1 * PW], F32, tag=f"pbuf{h}", name=f"pbuf{h}") for h in range(2)]
                ubuf = [pb.tile([128, PW], F32, tag=f"ubuf{h}", name=f"ubuf{h}") for h in range(2)]
                osb = [pb.tile([128, O], F32, tag=f"osb{h}", name=f"osb{h}") for h in range(2)]

                def load_p(buf, blk_expr):
                    for c in range(C1):
                        nc.sync.dma_start(buf[:, c * PW:(c + 1) * PW],
                                          p1d[c, :, blk_expr])

                def step(ph, ub, s):
                    # ---- layer 1 (z1/z1h/z1s/u1/a1 hold step-(t-1) values) ----
                    TT(a1[:], a1[:], rh1[:], op=AluOp.mult)
                    TT(tm1[:], z1[:], rm1[:], op=AluOp.mult)
                    TT(a1[:], a1[:], tm1[:], op=AluOp.add)
                    nc.scalar.activation(th1[:], a1[:], ACT_COPY, bias=B0, scale=BETA)
                    TT(tu1[:], u1[:], al1[:], op=AluOp.mult)
                    TT(tm1[:], z1[:], th1[:], op=AluOp.mult)
                    TT(tu1[:], tu1[:], tm1[:], op=AluOp.subtract)
                    pv = ph[:].rearrange("p (c s b) -> p c s b", c=C1, s=BLK)[:, :, s, :]
                    tu1v = tu1[:].rearrange("p (c b) -> p c b", c=C1)
                    TT(tu1v, tu1v, pv, op=AluOp.add)

                    ps1 = bps.tile([128, C1 * BC], F32, tag="ps1")
                    for m in range(C1):
                        dst = ps1[:, m * BC:(m + 1) * BC]
                        for k in range(C1):
                            nc.tensor.matmul(
                                dst,
                                w1h_hi_t[:, k * H1 + m * 128:k * H1 + (m + 1) * 128],
                                z1h[:, k * BC:(k + 1) * BC],
                                start=(k == 0), stop=False)
                        for k in range(C1):
                            nc.tensor.matmul(
                                dst,
                                w1h_lo_t[:, k * H1 + m * 128:k * H1 + (m + 1) * 128],
                                z1s[:, k * BC:(k + 1) * BC],
                                start=False, stop=(k == C1 - 1))
                    TT(u1[:], tu1[:], ps1[:], op=AluOp.add)
                    TT(z1h[:], u1[:], th1[:], op=AluOp.is_gt)
                    nc.vector.tensor_scalar_mul(z1s[:], z1h[:], LO)
                    nc.vector.tensor_copy(z1[:], z1h[:])

                    # ---- layer 2 (z1h/z1s now step-t; z2* step-(t-1)) ----
                    TT(a2[:], a2[:], rh2[:], op=AluOp.mult)
                    TT(tm2[:], z2[:], rm2[:], op=AluOp.mult)
                    TT(a2[:], a2[:], tm2[:], op=AluOp.add)
                    nc.scalar.activation(th2[:], a2[:], ACT_COPY, bias=B0, scale=BETA)
                    TT(tu2[:], u2[:], al2[:], op=AluOp.mult)
                    TT(tm2[:], z2[:], th2[:], op=AluOp.mult)
                    TT(tu2[:], tu2[:], tm2[:], op=AluOp.subtract)

                    ps2 = bps.tile([128, C2 * BC], F32, tag="ps2")
                    for m in range(C2):
                        dst = ps2[:, m * BC:(m + 1) * BC]
                        for k in range(KC2):
                            lhs = w2_hi_t[:, k * H2 + m * 128:k * H2 + (m + 1) * 128]
                            rhs = (z1h[:, k * BC:(k + 1) * BC] if k < C1
                                   else z2h[:, (k - C1) * BC:(k - C1 + 1) * BC])
                            nc.tensor.matmul(dst, lhs, rhs, start=(k == 0), stop=False)
                        for k in range(KC2):
                            lhs = w2_lo_t[:, k * H2 + m * 128:k * H2 + (m + 1) * 128]
                            rhs = (z1s[:, k * BC:(k + 1) * BC] if k < C1
                                   else z2s[:, (k - C1) * BC:(k - C1 + 1) * BC])
                            nc.tensor.matmul(dst, lhs, rhs, start=False,
                                             stop=(k == KC2 - 1))
                    TT(u2[:], tu2[:], ps2[:], op=AluOp.add)
                    TT(z2h[:], u2[:], th2[:], op=AluOp.is_gt)
                    nc.vector.tensor_scalar_mul(z2s[:], z2h[:], LO)
                    nc.vector.tensor_copy(z2[:], z2h[:])

                    # ---- output ----
                    pso = bps.tile([128, BC], F32, tag="pso")
                    for k in range(C2):
                        nc.tensor.matmul(pso[:], wo_hi_t[:, k * O:(k + 1) * O],
                                         z2h[:, k * BC:(k + 1) * BC],
                                         start=(k == 0), stop=False)
                    for k in range(C2):
                        nc.tensor.matmul(pso[:], wo_lo_t[:, k * O:(k + 1) * O],
                                         z2s[:, k * BC:(k + 1) * BC],
                                         start=False, stop=(k == C2 - 1))
                    STT(uo[:], uo[:], aop[:, 0:1], pso[:],
                        op0=AluOp.mult, op1=AluOp.add)
                    nc.vector.tensor_copy(ub[:, s * BC:(s + 1) * BC], uo[:])

                def flush_out(ub, h, row_expr):
                    for j in range(OTR):
                        pst = bps.tile([128, 128], F32, tag="pstB")
                        nc.tensor.transpose(pst[:], ub[:, j * 128:(j + 1) * 128],
                                            ident[:])
                        nc.vector.tensor_copy(osb[h][:], pst[:])
                        nc.sync.dma_start(out_rows[row_expr(j), :], osb[h][:])

                with tc.For_i(0, repsB, 1, name="scanrep") as _rb:
                    load_p(pbuf[0], bass.ds(0, PW))
                    with tc.For_i(0, NBLK // 2, 1, name="scan") as bi:
                        load_p(pbuf[1], bass.ds(bi * (2 * PW) + PW, PW))
                        for s in range(BLK):
                            step(pbuf[0], ubuf[0], s)
                        flush_out(ubuf[0], 0,
                                  lambda j: bass.ds(bi * (2 * PW) + j * 128, 128))
                        load_p(pbuf[0], bass.ds(bi * (2 * PW) + 2 * PW, PW))
                        for s in range(BLK):
                            step(pbuf[1], ubuf[1], s)
                        flush_out(ubuf[1], 1,
                                  lambda j: bass.ds(bi * (2 * PW) + PW + j * 128, 128))

                # ---------------- epilogue ----------------
                for name, st, nchunk in [("z1f", z1, C1), ("u1f", u1, C1),
                                         ("a1f", a1, C1), ("z2f", z2, C2),
                                         ("u2f", u2, C2), ("a2f", a2, C2),
                                         ("uof", uo, 1)]:
                    fsb = pb.tile([128, nchunk * 128], F32, tag="fsb")
                    for c in range(nchunk):
                        pst = bps.tile([128, 128], F32, tag="pstB")
                        nc.tensor.transpose(pst[:BC, :], st[:, c * BC:(c + 1) * BC],
                                            ident[:])
                        nc.vector.tensor_copy(fsb[:BC, c * 128:(c + 1) * 128],
                                              pst[:BC, :])
                    nc.sync.dma_start(fin[name][:, :], fsb[:BC, :])

    nc.compile()
    return nc


def prep_inputs(x, W1, W2, Wout, tau_m1, tau_adp1, tau_m2, tau_adp2, tau_m_out):
    """Host-side preprocessing shared by all cores (weights) + per-core x."""
    a1 = np.exp(-1.0 / np.asarray(tau_m1, np.float32)).astype(np.float32)
    r1 = np.exp(-1.0 / np.asarray(tau_adp1, np.float32)).astype(np.float32)
    a2 = np.exp(-1.0 / np.asarray(tau_m2, np.float32)).astype(np.float32)
    r2 = np.exp(-1.0 / np.asarray(tau_adp2, np.float32)).astype(np.float32)
    ao = np.exp(-1.0 / np.asarray(tau_m_out, np.float32)).astype(np.float32)

    W1 = np.asarray(W1, np.float32)
    W2 = np.asarray(W2, np.float32)
    Wout = np.asarray(Wout, np.float32)
    w1x = (W1[:I] * (1.0 - a1)[None, :]).astype(np.float32)
    w1h_hi, w1h_lo = split_fp16(W1[I:] * (1.0 - a1)[None, :])
    w2_hi, w2_lo = split_fp16(W2 * (1.0 - a2)[None, :])
    wo_hi, wo_lo = split_fp16(Wout * (1.0 - ao)[None, :])

    def bc_tile(v):  # [H] -> [128, C*BC] broadcast (col = 16*chunk + batch)
        C = v.shape[0] // 128
        return np.repeat(v.reshape(C, 128).T[:, :, None], BC, axis=2).reshape(128, C * BC).astype(np.float32)

    shared = {
        "w1x": w1x, "w1h_hi": w1h_hi, "w1h_lo": w1h_lo,
        "w2_hi": w2_hi, "w2_lo": w2_lo, "wo_hi": wo_hi, "wo_lo": wo_lo,
        "al1_bc": bc_tile(a1), "rh1_bc": bc_tile(r1),
        "rm1_bc": bc_tile(1.0 - r1),
        "al2_bc": bc_tile(a2), "rh2_bc": bc_tile(r2),
        "rm2_bc": bc_tile(1.0 - r2),
        "ao_pp": ao.reshape(128, 1).astype(np.float32),
    }
    return shared


_prog_cache = {}


def kernel(x, W1, W2, Wout, tau_m1, tau_adp1, tau_m2, tau_adp2, tau_m_out):
    x = np.asarray(x, np.float32)
    T = x.shape[0]
    shared = prep_inputs(x, W1, W2, Wout, tau_m1, tau_adp1, tau_m2,
                         tau_adp2, tau_m_out)

    key = T
    if key not in _prog_cache:
        _prog_cache[key] = build_program(T)
    nc = _prog_cache[key]

    in_maps = []
    for c in range(NCORES):
        xc = np.ascontiguousarray(x[:, c * BC:(c + 1) * BC, :]).reshape(T * BC, I)
        m = dict(shared)
        m["x_rows"] = xc
        in_maps.append(m)

    res = run_bass_kernel_spmd(nc, in_maps, core_ids=list(range(NCORES)))

    outputs = np.empty((T, B_FULL, O), np.float32)
    z1f = np.empty((B_FULL, H1), np.float32); u1f = np.empty_like(z1f)
    a1f = np.empty_like(z1f)
    z2f = np.empty((B_FULL, H2), np.float32); u2f = np.empty_like(z2f)
    a2f = np.empty_like(z2f)
    uof = np.empty((B_FULL, O), np.float32)
    for c in range(NCORES):
        r = res.results[c]
        outputs[:, c * BC:(c + 1) * BC, :] = r["out_rows"].reshape(T, BC, O)
        z1f[c * BC:(c + 1) * BC] = r["z1f"]; u1f[c * BC:(c + 1) * BC] = r["u1f"]
        a1f[c * BC:(c + 1) * BC] = r["a1f"]; z2f[c * BC:(c + 1) * BC] = r["z2f"]
        u2f[c * BC:(c + 1) * BC] = r["u2f"]; a2f[c * BC:(c + 1) * BC] = r["a2f"]
        uof[c * BC:(c + 1) * BC] = r["uof"]

    return (outputs, ((z1f, u1f, a1f, z2f, u2f, a2f), uof))
